# revision 5
# baseline (speedup 1.0000x reference)
"""DayAdapter Trainium2 kernel.

y[b] = softsign(x[b] @ W[day_ids[b]] + b[day_ids[b]])
  x: [64, 1024, 512] f32, W: [24, 512, 512] f32, b: [24, 512] f32,
  day_ids: [64] i64.

Strategy: data-parallel over batch (8 samples per NeuronCore, 8 cores).
Host side: gather W[day_ids] / b[day_ids] per shard and transpose x so the
contraction dim (d) lands on SBUF partitions (fp32 has no DMA-transpose
path on TRN2). Device side, per sample and per 128-row tile of x:
  - 4 accumulating PE matmuls (K=128, N=512, float32r full-rate)
  - 1 K=1 matmul (ones[1,128].T @ bias[1,512]) adds the per-day bias
  - ACT: a=|y| (PSUM read), a+=1
  - DVE: r=reciprocal_approx_fast(a); out = y * r (PSUM read)
  - DMA out
"""

import sys

if "/opt/trn_rl_repo" not in sys.path:
    sys.path.insert(0, "/opt/trn_rl_repo")

import numpy as np

import concourse.bacc as bacc
import concourse.mybir as mybir
import concourse.tile as tile
from concourse.bass import ts
from concourse.bass_utils import run_bass_kernel_spmd

N_CORES = 8
B = 64
T = 1024
D = 512
SAMPLES_PER_CORE = B // N_CORES  # 8
P = 128
KBLK = D // P  # 4 contraction blocks
TTILES = T // P  # 8 row tiles per sample

# float32r streams 1 row/cycle on the PE (vs 4 for float32) at ~2^-11
# mantissa rounding; fp32 fallback available for accuracy debugging.
USE_F32R = True

_CACHE = {}

# test.py reads this for exec_time_ns after a traced run.
LAST_RESULTS = None
TRACE = False


def _build(bench_reps=None):
    key = ("prog", USE_F32R, bench_reps)
    if key in _CACHE:
        return _CACHE[key]

    mm_dt = mybir.dt.float32r if USE_F32R else mybir.dt.float32
    f32 = mybir.dt.float32

    nc = bacc.Bacc("TRN2", debug=False, num_devices=N_CORES)
    xT = nc.dram_tensor("xT", [SAMPLES_PER_CORE, D, T], mm_dt, kind="ExternalInput").ap()
    Wg = nc.dram_tensor("Wg", [SAMPLES_PER_CORE, D, D], mm_dt, kind="ExternalInput").ap()
    bg = nc.dram_tensor("bg", [SAMPLES_PER_CORE, 1, D], mm_dt, kind="ExternalInput").ap()
    ones = nc.dram_tensor("ones", [1, P], mm_dt, kind="ExternalInput").ap()
    y = nc.dram_tensor("y", [SAMPLES_PER_CORE, T, D], f32, kind="ExternalOutput").ap()

    with tile.TileContext(nc) as tc:
        with (
            tc.tile_pool(name="xt", bufs=2) as xt_pool,
            tc.tile_pool(name="w", bufs=2) as w_pool,
            tc.tile_pool(name="bias", bufs=2) as b_pool,
            tc.tile_pool(name="const", bufs=1) as c_pool,
            tc.tile_pool(name="work", bufs=4) as work_pool,
            tc.tile_pool(name="out", bufs=4) as out_pool,
            tc.tile_pool(name="psum", bufs=4, space="PSUM") as psum_pool,
        ):
            import contextlib

            ones_sb = c_pool.tile([1, P], mm_dt)
            nc.sync.dma_start(ones_sb[:], ones[:])

            loop_cm = (
                tc.For_i(
                    0,
                    bench_reps,
                    1,
                    hint_engines=(
                        mybir.EngineType.PE,
                        mybir.EngineType.Activation,
                        mybir.EngineType.DVE,
                        mybir.EngineType.SP,
                    ),
                )
                if bench_reps
                else contextlib.nullcontext()
            )
            with loop_cm:
              for s in range(SAMPLES_PER_CORE):
                xt_sb = xt_pool.tile([P, KBLK, T], mm_dt, tag="xt")
                nc.sync.dma_start(
                    xt_sb[:], xT[s].rearrange("(o p) t -> p o t", p=P)
                )
                w_sb = w_pool.tile([P, KBLK, D], mm_dt, tag="w")
                nc.sync.dma_start(
                    w_sb[:], Wg[s].rearrange("(o p) e -> p o e", p=P)
                )
                bias_sb = b_pool.tile([1, D], mm_dt, tag="bias")
                nc.sync.dma_start(bias_sb[:], bg[s])

                for j in range(TTILES):
                    acc = psum_pool.tile([P, D], f32, tag="acc")
                    for k in range(KBLK):
                        nc.tensor.matmul(
                            acc[:],
                            xt_sb[:, k, ts(j, P)],
                            w_sb[:, k, :],
                            start=(k == 0),
                            stop=False,
                        )
                    nc.tensor.matmul(
                        acc[:],
                        ones_sb[:],
                        bias_sb[:],
                        start=False,
                        stop=True,
                    )

                    # softsign: out = y / (1 + |y|)
                    den = work_pool.tile([P, D], f32, tag="den")
                    nc.scalar.activation(
                        den[:], acc[:], mybir.ActivationFunctionType.Abs
                    )
                    nc.scalar.add(den[:], den[:], 1.0)
                    rec = work_pool.tile([P, D], f32, tag="rec")
                    nc.vector.reciprocal_approx_fast(rec[:], den[:])
                    out_sb = out_pool.tile([P, D], f32, tag="out")
                    nc.vector.tensor_mul(out_sb[:], acc[:], rec[:])
                    nc.sync.dma_start(y[s, ts(j, P), :], out_sb[:])

    nc.compile()
    _CACHE[key] = nc
    return nc


def kernel(x, day_ids, W, b):
    global LAST_RESULTS
    x = np.ascontiguousarray(x, dtype=np.float32)
    W = np.asarray(W, dtype=np.float32)
    b = np.asarray(b, dtype=np.float32)
    ids = np.asarray(day_ids).astype(np.int64)

    # host-side shard prep: per-sample transpose of x, gather of W/b
    xT = np.ascontiguousarray(x.transpose(0, 2, 1))  # [B, D, T]
    Wg = np.ascontiguousarray(W[ids])  # [B, D, D]
    bg = np.ascontiguousarray(b[ids]).reshape(B, 1, D)
    ones = np.ones((1, P), dtype=np.float32)

    nc = _build()
    in_maps = []
    for c in range(N_CORES):
        lo, hi = c * SAMPLES_PER_CORE, (c + 1) * SAMPLES_PER_CORE
        in_maps.append(
            {"xT": xT[lo:hi], "Wg": Wg[lo:hi], "bg": bg[lo:hi], "ones": ones}
        )

    res = run_bass_kernel_spmd(
        nc, in_maps, core_ids=list(range(N_CORES)), trace=TRACE
    )
    LAST_RESULTS = res
    out = np.concatenate([res.results[c]["y"] for c in range(N_CORES)], axis=0)
    return out.astype(np.float32)


# revision 6
# speedup vs baseline: 1.7464x; 1.7464x over previous
"""DayAdapter Trainium2 kernel.

y[b] = softsign(x[b] @ W[day_ids[b]] + b[day_ids[b]])
  x: [64, 1024, 512] f32, W: [24, 512, 512] f32, b: [24, 512] f32,
  day_ids: [64] i64.

Strategy: data-parallel over batch (8 samples per NeuronCore, 8 cores).
Host side: gather W[day_ids] / b[day_ids] per shard and transpose x so the
contraction dim (d) lands on SBUF partitions (fp32 has no DMA-transpose
path on TRN2). Device side, per sample and per 128-row tile of x:
  - 4 accumulating PE matmuls (K=128, N=512, float32r full-rate)
  - 1 K=1 matmul (ones[1,128].T @ bias[1,512]) adds the per-day bias
  - ACT: a=|y| (PSUM read), a+=1
  - DVE: r=reciprocal_approx_fast(a); out = y * r (PSUM read)
  - DMA out
"""

import sys

if "/opt/trn_rl_repo" not in sys.path:
    sys.path.insert(0, "/opt/trn_rl_repo")

import numpy as np

import concourse.bacc as bacc
import concourse.mybir as mybir
import concourse.tile as tile
from concourse.bass import ts
from concourse.bass_utils import run_bass_kernel_spmd

N_CORES = 8
B = 64
T = 1024
D = 512
SAMPLES_PER_CORE = B // N_CORES  # 8
P = 128
KBLK = D // P  # 4 contraction blocks
TTILES = T // P  # 8 row tiles per sample

# float32r streams 1 row/cycle on the PE (vs 4 for float32) at ~2^-11
# mantissa rounding; fp32 fallback available for accuracy debugging.
USE_F32R = True

_CACHE = {}

# test.py reads this for exec_time_ns after a traced run.
LAST_RESULTS = None
TRACE = False


def _build(bench_reps=None):
    key = ("prog", USE_F32R, bench_reps)
    if key in _CACHE:
        return _CACHE[key]

    mm_dt = mybir.dt.float32r if USE_F32R else mybir.dt.float32
    f32 = mybir.dt.float32

    nc = bacc.Bacc("TRN2", debug=False, num_devices=N_CORES)
    xT = nc.dram_tensor("xT", [SAMPLES_PER_CORE, D, T], mm_dt, kind="ExternalInput").ap()
    Wg = nc.dram_tensor("Wg", [SAMPLES_PER_CORE, D, D], mm_dt, kind="ExternalInput").ap()
    bg = nc.dram_tensor("bg", [SAMPLES_PER_CORE, 1, D], mm_dt, kind="ExternalInput").ap()
    ones = nc.dram_tensor("ones", [1, P], mm_dt, kind="ExternalInput").ap()
    y = nc.dram_tensor("y", [SAMPLES_PER_CORE, T, D], f32, kind="ExternalOutput").ap()

    OB = 2  # t-tiles per output DMA

    with tile.TileContext(nc) as tc:
        with (
            tc.tile_pool(name="xt", bufs=3) as xt_pool,
            tc.tile_pool(name="w", bufs=3) as w_pool,
            tc.tile_pool(name="bias", bufs=3) as b_pool,
            tc.tile_pool(name="const", bufs=1) as c_pool,
            tc.tile_pool(name="work", bufs=4) as work_pool,
            tc.tile_pool(name="out", bufs=6) as out_pool,
            tc.tile_pool(name="psum", bufs=4, space="PSUM") as psum_pool,
        ):
            import contextlib

            ones_sb = c_pool.tile([1, P], mm_dt)
            nc.sync.dma_start(ones_sb[:], ones[:])

            loop_cm = (
                tc.For_i(
                    0,
                    bench_reps,
                    1,
                    hint_engines=(
                        mybir.EngineType.PE,
                        mybir.EngineType.Activation,
                        mybir.EngineType.DVE,
                        mybir.EngineType.SP,
                    ),
                )
                if bench_reps
                else contextlib.nullcontext()
            )
            with loop_cm:
                loaded = {}

                def load(s):
                    xt_sb = xt_pool.tile([P, KBLK, T], mm_dt, tag="xt")
                    nc.sync.dma_start(
                        xt_sb[:], xT[s].rearrange("(o p) t -> p o t", p=P)
                    )
                    w_sb = w_pool.tile([P, KBLK, D], mm_dt, tag="w")
                    nc.sync.dma_start(
                        w_sb[:], Wg[s].rearrange("(o p) e -> p o e", p=P)
                    )
                    bias_sb = b_pool.tile([1, D], mm_dt, tag="bias")
                    nc.sync.dma_start(bias_sb[:], bg[s])
                    loaded[s] = (xt_sb, w_sb, bias_sb)

                load(0)
                if SAMPLES_PER_CORE > 1:
                    load(1)
                for s in range(SAMPLES_PER_CORE):
                    xt_sb, w_sb, bias_sb = loaded.pop(s)

                    for jb in range(TTILES // OB):
                        if jb == 1 and s + 2 < SAMPLES_PER_CORE:
                            load(s + 2)
                        outs = out_pool.tile([P, OB, D], f32, tag="out")
                        for jj in range(OB):
                            j = jb * OB + jj
                            acc = psum_pool.tile([P, D], f32, tag="acc")
                            for k in range(KBLK):
                                nc.tensor.matmul(
                                    acc[:],
                                    xt_sb[:, k, ts(j, P)],
                                    w_sb[:, k, :],
                                    start=(k == 0),
                                    stop=False,
                                )
                            nc.tensor.matmul(
                                acc[:],
                                ones_sb[:],
                                bias_sb[:],
                                start=False,
                                stop=True,
                            )

                            # softsign: out = y / (1 + |y|)
                            den = work_pool.tile([P, D], f32, tag="den")
                            nc.scalar.activation(
                                den[:], acc[:], mybir.ActivationFunctionType.Abs
                            )
                            nc.scalar.add(den[:], den[:], 1.0)
                            rec = work_pool.tile([P, D], f32, tag="rec")
                            nc.vector.reciprocal_approx_fast(rec[:], den[:])
                            nc.vector.tensor_mul(outs[:, jj, :], acc[:], rec[:])
                        nc.gpsimd.dma_start(
                            y[s].rearrange("(b p) e -> p b e", p=P)[
                                :, jb * OB : (jb + 1) * OB, :
                            ],
                            outs[:],
                        )

    nc.compile()
    _CACHE[key] = nc
    return nc


def kernel(x, day_ids, W, b):
    global LAST_RESULTS
    x = np.ascontiguousarray(x, dtype=np.float32)
    W = np.asarray(W, dtype=np.float32)
    b = np.asarray(b, dtype=np.float32)
    ids = np.asarray(day_ids).astype(np.int64)

    # host-side shard prep: per-sample transpose of x, gather of W/b
    xT = np.ascontiguousarray(x.transpose(0, 2, 1))  # [B, D, T]
    Wg = np.ascontiguousarray(W[ids])  # [B, D, D]
    bg = np.ascontiguousarray(b[ids]).reshape(B, 1, D)
    ones = np.ones((1, P), dtype=np.float32)

    nc = _build()
    in_maps = []
    for c in range(N_CORES):
        lo, hi = c * SAMPLES_PER_CORE, (c + 1) * SAMPLES_PER_CORE
        in_maps.append(
            {"xT": xT[lo:hi], "Wg": Wg[lo:hi], "bg": bg[lo:hi], "ones": ones}
        )

    res = run_bass_kernel_spmd(
        nc, in_maps, core_ids=list(range(N_CORES)), trace=TRACE
    )
    LAST_RESULTS = res
    out = np.concatenate([res.results[c]["y"] for c in range(N_CORES)], axis=0)
    return out.astype(np.float32)
